# revision 2
# baseline (speedup 1.0000x reference)
"""Trainium2 Bass kernel for nn_BTD_AutoCorrelationLayer.

Math (reference):
  q = (queries @ Wq + bq).reshape(H, B, L, DH)   # raw reshape, scrambled
  full[b,i,j,k] = sum_{n,h} core[n,h]/H * q[n,b,i,h]*k[n,b,j,h]*v[n,b,k,h]
  out = full.reshape(B, L, S*S) @ Wo + bo

Key identities used:
  q[n,b,r*8+chi,h] = QP[n*384 + b*12 + r, chi*64 + h]  where QP = queries_flat @ Wq
  -> batch b only needs projection rows {n*384+b*12+r}, so data-parallel over b
     across 8 cores needs only 384 projection rows per core (no collectives).
  The (i,j,k) labels can be consistently permuted (i' = chi*12+r) if Wo rows are
  pre-permuted and output rows un-permuted on host.

Per core (b in [4c,4c+4)):
  1. PE: X^T projections -> PSUM [c-cols, rows]; each chi's 64-col weight block
     is matmul'd twice (into both PSUM halves) so the rearrange copies below
     stay partition-aligned while weights ship in natural (non-dup) layout.
  2. ACT: rearrange PSUM -> CQT/K2/V2 [nh-chunk 128, (b, i') 384] (K scaled by
     core/H so Q needs no extra pass on the critical path).
  3. DVE/Pool/ACT: KV[nh, (b, j'k')] = K2*V2 Khatri-Rao via broadcast APs.
  4. PE: full^T[jk-chunk 128, (b,i') 384] += KV_chunk^T-slices @ CQT  (PSUM acc).
  5. PE: out^T[d-tile 128, (b,i') 384] += Wo_chunk @ full_chunk      (PSUM acc).
"""

import numpy as np

B, L, S, D, H, DH = 32, 96, 96, 512, 8, 64
NC = 8
BPC = B // NC          # 4 batches per core
RPB = 12               # projection rows per (n, b) block
ROWS = H * BPC * RPB   # 384 rows per core
JK = S * S             # 9216
NCH = 4                # nh chunks of 128 (= 2 heads)
JSUP = 4               # j' per super-block -> 384 jk = 3 psum chunks
NSUP = S // JSUP       # 24 supers
CHUNKS = JK // 128     # 72


def _kv_roles(nd=57, np_=30, na=9):
    """Bresenham-interleave D/P/A roles over the 96 KV chunks."""
    roles = []
    cd = cp = ca = 0
    for i in range(96):
        # which role is most behind its quota?
        scores = (
            ((i + 1) * nd / 96 - cd, "D"),
            ((i + 1) * np_ / 96 - cp, "P"),
            ((i + 1) * na / 96 - ca, "A"),
        )
        r = max(scores)[1]
        roles.append(r)
        cd += r == "D"
        cp += r == "P"
        ca += r == "A"
    return roles


KV_ROLE = _kv_roles()

_CACHE = {}


def _build():
    from contextlib import ExitStack
    import concourse.bass as bass
    import concourse.mybir as mybir
    import concourse.tile as tile
    from concourse import bacc

    f32 = mybir.dt.float32
    bf16 = mybir.dt.bfloat16
    COPY = mybir.ActivationFunctionType.Copy
    IDENT = mybir.ActivationFunctionType.Identity

    nc = bacc.Bacc("TRN2", target_bir_lowering=False, debug=False,
                   num_devices=NC)

    # DRAM I/O (per-core tensors; weights replicated in each core's map)
    qt_d = nc.dram_tensor("qt", [D, ROWS], bf16, kind="ExternalInput")
    kt_d = nc.dram_tensor("kt", [D, ROWS], bf16, kind="ExternalInput")
    vt_d = nc.dram_tensor("vt", [D, ROWS], bf16, kind="ExternalInput")
    # natural (in, out) layout, no duplication: [D, 512]
    wq_d = nc.dram_tensor("wq", [D, 512], bf16, kind="ExternalInput")
    wk_d = nc.dram_tensor("wk", [D, 512], bf16, kind="ExternalInput")
    wv_d = nc.dram_tensor("wv", [D, 512], bf16, kind="ExternalInput")
    wo_d = nc.dram_tensor("wo", [JK, D], bf16, kind="ExternalInput")
    # packed consts: cols 0:4 cs2, 4:12 bq, 12:20 bk, 20:28 bv, 28:32 bo
    cst_d = nc.dram_tensor("cst", [128, 32], f32, kind="ExternalInput")
    out_d = nc.dram_tensor("outT", [D, ROWS], bf16, kind="ExternalOutput")

    with tile.TileContext(nc) as tc, ExitStack() as ctx:
        P = ctx.enter_context
        const = P(tc.tile_pool(name="const", bufs=1))
        big = P(tc.tile_pool(name="big", bufs=1))
        kvp = P(tc.tile_pool(name="kv", bufs=12))
        psmm = P(tc.tile_pool(name="psmm", bufs=4, space="PSUM"))
        psout = P(tc.tile_pool(name="psout", bufs=1, space="PSUM"))

        # ---- warmup tile (no DMA dependence; keeps PE p-state ramped while
        # input DMAs stream)
        warm = const.tile([128, 512], bf16, tag="warm")
        nc.vector.memset(warm[:], 0.125)

        # ---- DMA order: consts, then k, v, q (x then w per name), then Wo.
        cst = const.tile([128, 32], f32, tag="cst")
        nc.sync.dma_start(cst[:], cst_d[:])
        cs2 = cst[:, 0:4]
        bias = {"q": cst[:, 4:12], "k": cst[:, 12:20], "v": cst[:, 20:28]}
        bor = cst[:, 28:32]

        xt, w2 = {}, {}
        for name, xd, wd in (("k", kt_d, wk_d), ("v", vt_d, wv_d),
                             ("q", qt_d, wq_d)):
            tx = big.tile([128, 4 * ROWS], bf16, tag=f"xt_{name}",
                          name=f"xt_{name}")
            tw = big.tile([128, 4 * 512], bf16, tag=f"w_{name}",
                          name=f"w_{name}")
            nc.sync.dma_start(
                tx[:].rearrange("p (dc r) -> p dc r", dc=4),
                xd[:].rearrange("(dc p) r -> p dc r", dc=4))
            nc.sync.dma_start(
                tw[:].rearrange("p (dc c) -> p dc c", dc=4),
                wd[:].rearrange("(dc p) c -> p dc c", dc=4))
            xt[name], w2[name] = tx, tw

        wo = big.tile([128, CHUNKS * 512], bf16, tag="wo")
        # first 8 chunks individually (fine-grained availability), rest in
        # 8-chunk batches
        for c in range(8):
            nc.sync.dma_start(wo[:, c * 512:(c + 1) * 512],
                              wo_d[c * 128:(c + 1) * 128, :])
        for g in range(1, 9):
            sl = wo[:, g * 8 * 512:(g + 1) * 8 * 512]
            nc.sync.dma_start(
                sl.rearrange("p (c d) -> p c d", c=8),
                wo_d[g * 1024:(g + 1) * 1024, :].rearrange(
                    "(c p) d -> p c d", c=8))

        t2 = {n: big.tile([128, NCH * 384], bf16, tag=f"t2_{n}",
                          name=f"t2_{n}")
              for n in ("q", "k", "v")}
        full = big.tile([128, CHUNKS * 384], bf16, tag="full")
        outs = big.tile([128, 4 * 384], bf16, tag="outs")

        # ---- output psum tiles (live for whole kernel)
        pout = [psout.tile([128, 384], f32, tag=f"po{i}", name=f"po{i}")
                for i in range(4)]

        # ---- PE warmup chain: garbage matmuls into pout (overwritten by the
        # real accumulation, which restarts with start=True)
        for i in range(6):
            nc.tensor.matmul(pout[i % 4][:], warm[:, 0:128],
                             warm[:, 0:384], start=True, stop=True)

        # ---- projections + rearrange, name-sequential (k, v, q) to match
        # DMA arrival; each chi's weight block hits both PSUM halves.
        for name in ("k", "v", "q"):
            # gated warmup: touch freshly-landed x to bridge PE idle gaps
            nc.tensor.matmul(pout[3][:], warm[:, 0:128],
                             xt[name][:, 0:384], start=True, stop=True)
            for chi in range(8):
                p = psmm.tile([128, 512], f32, tag="mm")
                for dc in range(4):
                    for npar in range(2):
                        nc.tensor.matmul(
                            p[npar * 64:npar * 64 + 64, 0:ROWS],
                            w2[name][:, dc * 512 + chi * 64:
                                     dc * 512 + chi * 64 + 64],
                            xt[name][:, dc * ROWS:(dc + 1) * ROWS],
                            start=(dc == 0), stop=(dc == 3))
                bt = bias[name]
                for npar in range(2):
                    src = p[npar * 64:npar * 64 + 64, 0:ROWS].rearrange(
                        "p (t u b r) -> p t u b r", t=4, u=2, b=BPC
                    )[:, :, npar, :, :]
                    dst = t2[name][npar * 64:npar * 64 + 64, :].rearrange(
                        "p (m b c r) -> p m b c r", m=NCH, b=BPC, c=8
                    )[:, :, :, chi, :]
                    nc.scalar.activation(
                        dst, src, IDENT,
                        bias=bt[npar * 64:npar * 64 + 64, chi:chi + 1],
                        scale=1.0)
            if name == "k":
                # fold core/H into K (K lands first; keeps Q off the
                # critical path)
                for m in range(NCH):
                    sl = t2["k"][:, m * 384:(m + 1) * 384]
                    nc.scalar.activation(sl, sl, COPY,
                                         scale=cs2[:, m:m + 1])

        # ---- supers: KV build -> contraction -> copy -> output matmul
        for js in range(NSUP):
            kvt = []
            for m in range(NCH):
                kv = kvp.tile([128, BPC * JSUP * 96], bf16, tag="kv")
                k2 = t2["k"][:, m * 384:(m + 1) * 384].rearrange(
                    "p (b j) -> p b j", b=BPC)[:, :, js * JSUP:(js + 1) * JSUP]
                v2 = t2["v"][:, m * 384:(m + 1) * 384].rearrange(
                    "p (b k) -> p b k", b=BPC)
                role = KV_ROLE[js * NCH + m]
                if role == "A":
                    # ACT as third KV producer: per (b, j) copy of V row-scaled
                    # by the K column. Scale APs must be fp32, so gather the
                    # 16 needed K columns into a small fp32 tile first.
                    k2f = kvp.tile([128, BPC * JSUP], f32, tag="k2f",
                                   name="k2f", bufs=3)
                    nc.vector.tensor_copy(
                        k2f[:].rearrange("p (b j) -> p b j", b=BPC),
                        k2)
                    for b in range(BPC):
                        for jj in range(JSUP):
                            nc.scalar.activation(
                                kv[:, (b * JSUP + jj) * 96:
                                   (b * JSUP + jj) * 96 + 96],
                                t2["v"][:, m * 384 + b * 96:
                                        m * 384 + b * 96 + 96],
                                COPY,
                                scale=k2f[:, b * JSUP + jj:
                                          b * JSUP + jj + 1])
                else:
                    eng = nc.vector if role == "D" else nc.gpsimd
                    eng.tensor_mul(
                        kv[:].rearrange("p (b j k) -> p b j k", b=BPC, j=JSUP),
                        k2.unsqueeze(3).broadcast_to((128, BPC, JSUP, 96)),
                        v2.unsqueeze(2).broadcast_to((128, BPC, JSUP, 96)))
                kvt.append(kv)
            for cj in range(3):
                c = js * 3 + cj
                p = psmm.tile([128, 512], f32, tag="mm")
                for b in range(BPC):
                    for m in range(NCH):
                        nc.tensor.matmul(
                            p[:, b * 96:(b + 1) * 96],
                            kvt[m][:, b * 384 + cj * 128:
                                   b * 384 + cj * 128 + 128],
                            t2["q"][:, m * 384 + b * 96:
                                    m * 384 + b * 96 + 96],
                            start=(m == 0), stop=(m == NCH - 1))
                nc.scalar.activation(full[:, c * 384:(c + 1) * 384],
                                     p[:, 0:384], COPY)
                for dt_ in range(4):
                    nc.tensor.matmul(
                        pout[dt_][:],
                        wo[:, c * 512 + dt_ * 128:c * 512 + dt_ * 128 + 128],
                        full[:, c * 384:(c + 1) * 384],
                        start=(c == 0), stop=(c == CHUNKS - 1))

        # ---- bias + store (per-dt so dt0 streams out while dt1-3 finish)
        for dt_ in range(4):
            nc.scalar.activation(outs[:, dt_ * 384:(dt_ + 1) * 384],
                                 pout[dt_][:], IDENT,
                                 bias=bor[:, dt_:dt_ + 1])
            nc.sync.dma_start(out_d[dt_ * 128:(dt_ + 1) * 128, :],
                              outs[:, dt_ * 384:(dt_ + 1) * 384])

    nc.compile()
    return nc


def _prep(queries, keys, values, Wq, bq, Wk, bk, Wv, bv, core, Wo, bo):
    import ml_dtypes
    bf16 = ml_dtypes.bfloat16
    f32 = np.float32

    # device row i' holds reference row i = imap[i'] = (i'%12)*8 + i'//12
    imap = np.array([(ip % 12) * 8 + ip // 12 for ip in range(96)])

    CS = (core.astype(f32) / H)                       # [H, DH]
    cst = np.zeros((128, 32), dtype=f32)
    for m in range(4):                                # cs2
        cst[:64, m] = CS[2 * m]
        cst[64:, m] = CS[2 * m + 1]
    cst[:, 4:12] = np.tile(bq.reshape(8, 64).T, (2, 1))   # [128, 8] (h,chi)
    cst[:, 12:20] = np.tile(bk.reshape(8, 64).T, (2, 1))
    cst[:, 20:28] = np.tile(bv.reshape(8, 64).T, (2, 1))
    cst[:, 28:32] = bo.astype(f32).reshape(4, 128).T      # bor[p, dt]

    Wo_r = Wo.astype(f32).reshape(S, S, D)
    Wo_p = np.ascontiguousarray(
        Wo_r[np.ix_(imap, imap)].reshape(JK, D)).astype(bf16)

    shared = dict(wq=np.ascontiguousarray(Wq).astype(bf16),
                  wk=np.ascontiguousarray(Wk).astype(bf16),
                  wv=np.ascontiguousarray(Wv).astype(bf16),
                  wo=Wo_p, cst=cst)

    qf = queries.reshape(B * L, D)
    kf = keys.reshape(B * S, D)
    vf = values.reshape(B * S, D)
    n_i, b_i, r_i = np.meshgrid(np.arange(H), np.arange(BPC), np.arange(RPB),
                                indexing="ij")
    maps = []
    for c in range(NC):
        idx = (n_i * 384 + 48 * c + b_i * 12 + r_i).reshape(-1)
        m = dict(shared)
        m["qt"] = np.ascontiguousarray(qf[idx].T).astype(bf16)
        m["kt"] = np.ascontiguousarray(kf[idx].T).astype(bf16)
        m["vt"] = np.ascontiguousarray(vf[idx].T).astype(bf16)
        maps.append(m)
    return maps, imap


def kernel(queries, keys, values, attn_mask, Wq, bq, Wk, bk, Wv, bv, core,
           Wo, bo, _want_trace=False):
    from concourse import bass_utils

    if "nc" not in _CACHE:
        _CACHE["nc"] = _build()
    nc = _CACHE["nc"]

    maps, imap = _prep(np.asarray(queries), np.asarray(keys),
                       np.asarray(values), np.asarray(Wq),
                       np.asarray(bq), np.asarray(Wk), np.asarray(bk),
                       np.asarray(Wv), np.asarray(bv), np.asarray(core),
                       np.asarray(Wo), np.asarray(bo))
    try:
        res = bass_utils.run_bass_kernel_spmd(
            nc, maps, core_ids=list(range(NC)), trace=_want_trace)
    except ModuleNotFoundError:
        res = bass_utils.run_bass_kernel_spmd(
            nc, maps, core_ids=list(range(NC)), trace=False)
    out = np.empty((B, L, D), dtype=np.float32)
    for c in range(NC):
        oT = np.asarray(res.results[c]["outT"], dtype=np.float32)  # [D, 384]
        o = oT.T.reshape(BPC, 96, D)          # rows in device i' order
        ref = np.empty((BPC, 96, D), dtype=np.float32)
        ref[:, imap, :] = o
        out[4 * c:4 * c + 4] = ref
    if _want_trace:
        _CACHE["last_results"] = res
    return out


# revision 7
# speedup vs baseline: 1.0698x; 1.0698x over previous
"""Trainium2 Bass kernel for nn_BTD_AutoCorrelationLayer.

Math (reference):
  q = (queries @ Wq + bq).reshape(H, B, L, DH)   # raw reshape, scrambled
  full[b,i,j,k] = sum_{n,h} core[n,h]/H * q[n,b,i,h]*k[n,b,j,h]*v[n,b,k,h]
  out = full.reshape(B, L, S*S) @ Wo + bo

Key identities used:
  q[n,b,r*8+chi,h] = QP[n*384 + b*12 + r, chi*64 + h]  where QP = queries_flat @ Wq
  -> batch b only needs projection rows {n*384+b*12+r}, so data-parallel over b
     across 8 cores needs only 384 projection rows per core (no collectives).
  The (i,j,k) labels can be consistently permuted (i' = chi*12+r) if Wo rows are
  pre-permuted and output rows un-permuted on host.

Per core (b in [4c,4c+4)):
  1. PE: X^T projections -> PSUM [c-cols, rows]; each chi's 64-col weight block
     is matmul'd twice (into both PSUM halves) so the rearrange copies below
     stay partition-aligned while weights ship in natural (non-dup) layout.
  2. ACT: rearrange PSUM -> CQT/K2/V2 [nh-chunk 128, (b, i') 384] (K scaled by
     core/H so Q needs no extra pass on the critical path).
  3. DVE/Pool/ACT: KV[nh, (b, j'k')] = K2*V2 Khatri-Rao via broadcast APs.
  4. PE: full^T[jk-chunk 128, (b,i') 384] += KV_chunk^T-slices @ CQT  (PSUM acc).
  5. PE: out^T[d-tile 128, (b,i') 384] += Wo_chunk @ full_chunk      (PSUM acc).
"""

import numpy as np

B, L, S, D, H, DH = 32, 96, 96, 512, 8, 64
NC = 8
BPC = B // NC          # 4 batches per core
RPB = 12               # projection rows per (n, b) block
ROWS = H * BPC * RPB   # 384 rows per core
JK = S * S             # 9216
NCH = 4                # nh chunks of 128 (= 2 heads)
JSUP = 4               # j' per super-block -> 384 jk = 3 psum chunks
NSUP = S // JSUP       # 24 supers
CHUNKS = JK // 128     # 72


def _kv_roles(nd=57, np_=30, na=9):
    """Bresenham-interleave D/P/A roles over the 96 KV chunks."""
    roles = []
    cd = cp = ca = 0
    for i in range(96):
        # which role is most behind its quota?
        scores = (
            ((i + 1) * nd / 96 - cd, "D"),
            ((i + 1) * np_ / 96 - cp, "P"),
            ((i + 1) * na / 96 - ca, "A"),
        )
        r = max(scores)[1]
        roles.append(r)
        cd += r == "D"
        cp += r == "P"
        ca += r == "A"
    return roles


KV_ROLE = _kv_roles()

_CACHE = {}


def _build():
    from contextlib import ExitStack
    import concourse.bass as bass
    import concourse.mybir as mybir
    import concourse.tile as tile
    from concourse import bacc

    f32 = mybir.dt.float32
    bf16 = mybir.dt.bfloat16
    COPY = mybir.ActivationFunctionType.Copy
    IDENT = mybir.ActivationFunctionType.Identity

    nc = bacc.Bacc("TRN2", target_bir_lowering=False, debug=False,
                   num_devices=NC)

    # DRAM I/O (per-core tensors; weights replicated in each core's map)
    qt_d = nc.dram_tensor("qt", [D, ROWS], bf16, kind="ExternalInput")
    kt_d = nc.dram_tensor("kt", [D, ROWS], bf16, kind="ExternalInput")
    vt_d = nc.dram_tensor("vt", [D, ROWS], bf16, kind="ExternalInput")
    # weights with chi-64-blocks duplicated into 128-col tiles: [D, 8*128]
    wq_d = nc.dram_tensor("wq", [D, 1024], bf16, kind="ExternalInput")
    wk_d = nc.dram_tensor("wk", [D, 1024], bf16, kind="ExternalInput")
    wv_d = nc.dram_tensor("wv", [D, 1024], bf16, kind="ExternalInput")
    wo_d = nc.dram_tensor("wo", [JK, D], bf16, kind="ExternalInput")
    # packed consts: cols 0:4 cs2, 4:12 bq, 12:20 bk, 20:28 bv, 28:32 bo
    cst_d = nc.dram_tensor("cst", [128, 32], f32, kind="ExternalInput")
    out_d = nc.dram_tensor("outT", [D, ROWS], bf16, kind="ExternalOutput")

    with tile.TileContext(nc) as tc, ExitStack() as ctx:
        P = ctx.enter_context
        const = P(tc.tile_pool(name="const", bufs=1))
        big = P(tc.tile_pool(name="big", bufs=1))
        kvp = P(tc.tile_pool(name="kv", bufs=11))
        psmm = P(tc.tile_pool(name="psmm", bufs=4, space="PSUM"))
        psout = P(tc.tile_pool(name="psout", bufs=1, space="PSUM"))

        # ---- warmup tile (no DMA dependence; keeps PE p-state ramped while
        # input DMAs stream)
        warm = const.tile([128, 512], bf16, tag="warm")
        nc.vector.memset(warm[:], 0.125)

        # ---- DMA order: consts, then k, v, q (x then w per name), then Wo.
        cst = const.tile([128, 32], f32, tag="cst")
        nc.sync.dma_start(cst[:], cst_d[:])
        cs2 = cst[:, 0:4]
        bias = {"q": cst[:, 4:12], "k": cst[:, 12:20], "v": cst[:, 20:28]}
        bor = cst[:, 28:32]

        xt, w2 = {}, {}
        for name, xd, wd in (("k", kt_d, wk_d), ("v", vt_d, wv_d),
                             ("q", qt_d, wq_d)):
            tx = big.tile([128, 4 * ROWS], bf16, tag=f"xt_{name}",
                          name=f"xt_{name}")
            tw = big.tile([128, 4 * 1024], bf16, tag=f"w_{name}",
                          name=f"w_{name}")
            nc.sync.dma_start(
                tx[:].rearrange("p (dc r) -> p dc r", dc=4),
                xd[:].rearrange("(dc p) r -> p dc r", dc=4))
            nc.sync.dma_start(
                tw[:].rearrange("p (dc c) -> p dc c", dc=4),
                wd[:].rearrange("(dc p) c -> p dc c", dc=4))
            xt[name], w2[name] = tx, tw

        wo = big.tile([128, CHUNKS * 512], bf16, tag="wo")
        # first 8 chunks individually (fine-grained availability), rest in
        # 8-chunk batches
        for c in range(8):
            nc.sync.dma_start(wo[:, c * 512:(c + 1) * 512],
                              wo_d[c * 128:(c + 1) * 128, :])
        for g in range(1, 9):
            sl = wo[:, g * 8 * 512:(g + 1) * 8 * 512]
            nc.sync.dma_start(
                sl.rearrange("p (c d) -> p c d", c=8),
                wo_d[g * 1024:(g + 1) * 1024, :].rearrange(
                    "(c p) d -> p c d", c=8))

        t2 = {n: big.tile([128, NCH * 384], bf16, tag=f"t2_{n}",
                          name=f"t2_{n}")
              for n in ("q", "k", "v")}
        full = big.tile([128, CHUNKS * 384], bf16, tag="full")
        outs = big.tile([128, 4 * 384], bf16, tag="outs")

        # ---- output psum tiles (live for whole kernel)
        pout = [psout.tile([128, 384], f32, tag=f"po{i}", name=f"po{i}")
                for i in range(4)]

        # ---- PE warmup chain: garbage matmuls into pout (overwritten by the
        # real accumulation, which restarts with start=True)
        for i in range(6):
            nc.tensor.matmul(pout[i % 4][:], warm[:, 0:128],
                             warm[:, 0:384], start=True, stop=True)

        # ---- projections + rearrange, name-sequential (k, v, q) to match
        # DMA arrival; each chi's weight block hits both PSUM halves.
        for name in ("k", "v", "q"):
            # gated warmup: touch freshly-landed x to bridge PE idle gaps
            nc.tensor.matmul(pout[3][:], warm[:, 0:128],
                             xt[name][:, 0:384], start=True, stop=True)
            for chi in range(8):
                p = psmm.tile([128, 512], f32, tag="mm")
                for dc in range(4):
                    nc.tensor.matmul(
                        p[:, 0:ROWS],
                        w2[name][:, dc * 1024 + chi * 128:
                                 dc * 1024 + chi * 128 + 128],
                        xt[name][:, dc * ROWS:(dc + 1) * ROWS],
                        start=(dc == 0), stop=(dc == 3))
                bt = bias[name]
                for npar in range(2):
                    src = p[npar * 64:npar * 64 + 64, 0:ROWS].rearrange(
                        "p (t u b r) -> p t u b r", t=4, u=2, b=BPC
                    )[:, :, npar, :, :]
                    dst = t2[name][npar * 64:npar * 64 + 64, :].rearrange(
                        "p (m b c r) -> p m b c r", m=NCH, b=BPC, c=8
                    )[:, :, :, chi, :]
                    nc.scalar.activation(
                        dst, src, IDENT,
                        bias=bt[npar * 64:npar * 64 + 64, chi:chi + 1],
                        scale=1.0)
            if name == "k":
                # fold core/H into K (K lands first; keeps Q off the
                # critical path)
                for m in range(NCH):
                    sl = t2["k"][:, m * 384:(m + 1) * 384]
                    nc.scalar.activation(sl, sl, COPY,
                                         scale=cs2[:, m:m + 1])

        # ---- supers: KV build -> contraction -> copy -> output matmul
        for js in range(NSUP):
            kvt = []
            for m in range(NCH):
                kv = kvp.tile([128, BPC * JSUP * 96], bf16, tag="kv")
                k2 = t2["k"][:, m * 384:(m + 1) * 384].rearrange(
                    "p (b j) -> p b j", b=BPC)[:, :, js * JSUP:(js + 1) * JSUP]
                v2 = t2["v"][:, m * 384:(m + 1) * 384].rearrange(
                    "p (b k) -> p b k", b=BPC)
                role = KV_ROLE[js * NCH + m]
                if role == "A":
                    # ACT as third KV producer: per (b, j) copy of V row-scaled
                    # by the K column. Scale APs must be fp32, so gather the
                    # 16 needed K columns into a small fp32 tile first.
                    k2f = kvp.tile([128, BPC * JSUP], f32, tag="k2f",
                                   name="k2f", bufs=3)
                    nc.vector.tensor_copy(
                        k2f[:].rearrange("p (b j) -> p b j", b=BPC),
                        k2)
                    for b in range(BPC):
                        for jj in range(JSUP):
                            nc.scalar.activation(
                                kv[:, (b * JSUP + jj) * 96:
                                   (b * JSUP + jj) * 96 + 96],
                                t2["v"][:, m * 384 + b * 96:
                                        m * 384 + b * 96 + 96],
                                COPY,
                                scale=k2f[:, b * JSUP + jj:
                                          b * JSUP + jj + 1])
                else:
                    eng = nc.vector if role == "D" else nc.gpsimd
                    eng.tensor_mul(
                        kv[:].rearrange("p (b j k) -> p b j k", b=BPC, j=JSUP),
                        k2.unsqueeze(3).broadcast_to((128, BPC, JSUP, 96)),
                        v2.unsqueeze(2).broadcast_to((128, BPC, JSUP, 96)))
                kvt.append(kv)
            for cj in range(3):
                c = js * 3 + cj
                p = psmm.tile([128, 512], f32, tag="mm")
                for b in range(BPC):
                    for m in range(NCH):
                        nc.tensor.matmul(
                            p[:, b * 96:(b + 1) * 96],
                            kvt[m][:, b * 384 + cj * 128:
                                   b * 384 + cj * 128 + 128],
                            t2["q"][:, m * 384 + b * 96:
                                    m * 384 + b * 96 + 96],
                            start=(m == 0), stop=(m == NCH - 1))
                nc.scalar.activation(full[:, c * 384:(c + 1) * 384],
                                     p[:, 0:384], COPY)
                for dt_ in range(4):
                    nc.tensor.matmul(
                        pout[dt_][:],
                        wo[:, c * 512 + dt_ * 128:c * 512 + dt_ * 128 + 128],
                        full[:, c * 384:(c + 1) * 384],
                        start=(c == 0), stop=(c == CHUNKS - 1))

        # ---- bias + store (per-dt so dt0 streams out while dt1-3 finish)
        for dt_ in range(4):
            nc.scalar.activation(outs[:, dt_ * 384:(dt_ + 1) * 384],
                                 pout[dt_][:], IDENT,
                                 bias=bor[:, dt_:dt_ + 1])
            nc.sync.dma_start(out_d[dt_ * 128:(dt_ + 1) * 128, :],
                              outs[:, dt_ * 384:(dt_ + 1) * 384])

    nc.compile()
    return nc


def _prep(queries, keys, values, Wq, bq, Wk, bk, Wv, bv, core, Wo, bo):
    import ml_dtypes
    bf16 = ml_dtypes.bfloat16
    f32 = np.float32

    # device row i' holds reference row i = imap[i'] = (i'%12)*8 + i'//12
    imap = np.array([(ip % 12) * 8 + ip // 12 for ip in range(96)])

    CS = (core.astype(f32) / H)                       # [H, DH]
    cst = np.zeros((128, 32), dtype=f32)
    for m in range(4):                                # cs2
        cst[:64, m] = CS[2 * m]
        cst[64:, m] = CS[2 * m + 1]
    cst[:, 4:12] = np.tile(bq.reshape(8, 64).T, (2, 1))   # [128, 8] (h,chi)
    cst[:, 12:20] = np.tile(bk.reshape(8, 64).T, (2, 1))
    cst[:, 20:28] = np.tile(bv.reshape(8, 64).T, (2, 1))
    cst[:, 28:32] = bo.astype(f32).reshape(4, 128).T      # bor[p, dt]

    Wo_r = Wo.astype(f32).reshape(S, S, D)
    Wo_p = np.ascontiguousarray(
        Wo_r[np.ix_(imap, imap)].reshape(JK, D)).astype(bf16)

    # weights: duplicate each 64-col chi block into both halves of 128 tiles
    def dup(W):
        Wb = W.astype(f32).reshape(D, 8, 64)
        out = np.empty((D, 8, 128), dtype=f32)
        out[:, :, :64] = Wb
        out[:, :, 64:] = Wb
        return np.ascontiguousarray(out.reshape(D, 1024)).astype(bf16)

    shared = dict(wq=dup(Wq), wk=dup(Wk), wv=dup(Wv), wo=Wo_p, cst=cst)

    qf = queries.reshape(B * L, D)
    kf = keys.reshape(B * S, D)
    vf = values.reshape(B * S, D)
    n_i, b_i, r_i = np.meshgrid(np.arange(H), np.arange(BPC), np.arange(RPB),
                                indexing="ij")
    maps = []
    for c in range(NC):
        idx = (n_i * 384 + 48 * c + b_i * 12 + r_i).reshape(-1)
        m = dict(shared)
        m["qt"] = np.ascontiguousarray(qf[idx].T).astype(bf16)
        m["kt"] = np.ascontiguousarray(kf[idx].T).astype(bf16)
        m["vt"] = np.ascontiguousarray(vf[idx].T).astype(bf16)
        maps.append(m)
    return maps, imap


def kernel(queries, keys, values, attn_mask, Wq, bq, Wk, bk, Wv, bv, core,
           Wo, bo, _want_trace=False):
    from concourse import bass_utils

    if "nc" not in _CACHE:
        _CACHE["nc"] = _build()
    nc = _CACHE["nc"]

    maps, imap = _prep(np.asarray(queries), np.asarray(keys),
                       np.asarray(values), np.asarray(Wq),
                       np.asarray(bq), np.asarray(Wk), np.asarray(bk),
                       np.asarray(Wv), np.asarray(bv), np.asarray(core),
                       np.asarray(Wo), np.asarray(bo))
    try:
        res = bass_utils.run_bass_kernel_spmd(
            nc, maps, core_ids=list(range(NC)), trace=_want_trace)
    except ModuleNotFoundError:
        res = bass_utils.run_bass_kernel_spmd(
            nc, maps, core_ids=list(range(NC)), trace=False)
    out = np.empty((B, L, D), dtype=np.float32)
    for c in range(NC):
        oT = np.asarray(res.results[c]["outT"], dtype=np.float32)  # [D, 384]
        o = oT.T.reshape(BPC, 96, D)          # rows in device i' order
        ref = np.empty((BPC, 96, D), dtype=np.float32)
        ref[:, imap, :] = o
        out[4 * c:4 * c + 4] = ref
    if _want_trace:
        _CACHE["last_results"] = res
    return out
